# revision 4
# baseline (speedup 1.0000x reference)
"""Trainium2 Bass kernel for a ViT-style transformer block — v5 (pipelined, batched LN).

Reference semantics (B=16, N=577, D=768, H=12, DH=64, HID=3072):
    h   = LN(x) ; qkv = h @ qkv_w + qkv_b
    dp  = q k^T / sqrt(DH)           (per head)
    attn= softmax over the HEAD axis (axis=1 of (B,H,N,N))
    x   = x + (attn @ v) @ proj_w + proj_b
    h   = LN(x); x = x + gelu(h @ fc1_w + fc1_b) @ fc2_w + fc2_b

Distribution: pure data parallelism -- 16 batches over 8 NeuronCores,
2 batches per core, full weights replicated, no collectives.

v2 changes vs the baseline:
  * tokens padded 577 -> 584 (not 640); matmul moving dims trimmed to
    real token counts (chunks 320+264 / per-qc 128,128,128,128,72).
  * softmax sum over heads via an 11-op bf16 tensor_tensor add chain
    (2x DVE mode) instead of one strided tensor_reduce (no fast mode).
  * reciprocal computed in-place on bf16 S (allow_low_precision) so the
    E *= 1/S broadcast multiply also runs in 2x DVE mode.
  * double-buffered pools so qc iterations and batch phases pipeline;
    E and the MLP gh chunks share one 2-slot pool (disjoint lifetimes).
  * fc1 runs over all tokens per hid-tile (N=320/264, 288 matmuls vs
    720), after LN2+transpose of the whole batch; gelu in [128,320] ops.
  * LN2 hoisted before all gelu so ACT function-set loads (Sqrt/Exp/
    Gelu) happen ~4x per batch instead of ~16.
  * k-scale (1/8) folded into the k columns of the resident qkv_w.
  * PSUM-evacuation copies split between DVE and ACT (Copy needs no
    activation-table load).

HW quirks kept from the baseline (crash / corruption avoidance):
  * scores matmuls contract K=64 at tile_position (0,0) only: q/k live
    in a [64, 24, NP] base-0 tile; no mixing of row strips.
  * the moving operand of a matmul must be contiguous per partition:
    E stored [k, kt, h, q] so attn@v rhs is an unstrided slice.
"""

import sys
import time

if "/opt/trn_rl_repo" not in sys.path:
    sys.path.insert(0, "/opt/trn_rl_repo")

import numpy as np

B, N, D = 16, 577, 768
H, DH, HID = 12, 64, 3072
EPS = 1e-6
NCORES = 8
BPC = B // NCORES          # batches per core
P = 128
NT = 5                     # token tiles
NP = 580                   # padded tokens (4-aligned)
DT = D // P                # 6
HT = HID // P              # 24
NPH = 640                  # hT padding (full 128-wide transposes)
LAST_ROWS = N - 4 * P      # 65 real tokens in the last tile
LAST_W = NP - 4 * P        # 72 padded width of the last tile
TCH = (256, 324)           # token chunks (sum = NP, 128-aligned split)
TCO = (0, 256)

LAST_EXEC_NS = None

_BUILT = {}


def _build(flags, repeat=1):
    from contextlib import ExitStack

    import concourse.bass as bass
    from concourse import bacc
    import concourse.mybir as mybir
    import concourse.tile as tile
    from concourse.bass import ts, ds

    f32 = mybir.dt.float32
    bf16 = mybir.dt.bfloat16
    AF = mybir.ActivationFunctionType
    OP = mybir.AluOpType

    nc = bacc.Bacc(trn_type="TRN2", target_bir_lowering=False, debug=False,
                   enable_asserts=False)

    x_d = nc.dram_tensor("x", [BPC, N, D], f32, kind="ExternalInput").ap()
    qkvw_d = nc.dram_tensor("qkv_w", [D, 3 * D], f32, kind="ExternalInput").ap()
    qkvb_d = nc.dram_tensor("qkv_b", [3 * D], f32, kind="ExternalInput").ap()
    projw_d = nc.dram_tensor("proj_w", [D, D], f32, kind="ExternalInput").ap()
    projb_d = nc.dram_tensor("proj_b", [D], f32, kind="ExternalInput").ap()
    ln1g_d = nc.dram_tensor("ln1_g", [D], f32, kind="ExternalInput").ap()
    ln1b_d = nc.dram_tensor("ln1_b", [D], f32, kind="ExternalInput").ap()
    ln2g_d = nc.dram_tensor("ln2_g", [D], f32, kind="ExternalInput").ap()
    ln2b_d = nc.dram_tensor("ln2_b", [D], f32, kind="ExternalInput").ap()
    fc1w_d = nc.dram_tensor("fc1_w", [D, HID], f32, kind="ExternalInput").ap()
    fc1b_d = nc.dram_tensor("fc1_b", [HID], f32, kind="ExternalInput").ap()
    fc2w_d = nc.dram_tensor("fc2_w", [HID, D], f32, kind="ExternalInput").ap()
    fc2b_d = nc.dram_tensor("fc2_b", [D], f32, kind="ExternalInput").ap()
    out_d = nc.dram_tensor("out", [BPC, N, D], f32, kind="ExternalOutput").ap()

    def bcast(src1d):
        return bass.AP(tensor=src1d.tensor, offset=src1d.offset,
                       ap=[[0, P], src1d.ap[0]])

    with tile.TileContext(nc) as tc:
        with ExitStack() as ctx:
            # ---------------- resident weights (bf16) ----------------
            wpool = ctx.enter_context(tc.tile_pool(name="weights", bufs=1))
            singles = ctx.enter_context(tc.tile_pool(name="singles", bufs=1))

            qkvw = wpool.tile([P, DT, 3 * D], bf16)
            projw = wpool.tile([P, DT, D], bf16)
            fc1w = wpool.tile([P, DT, HID], bf16)
            fc2w = wpool.tile([P, HT, D], bf16)

            cast_engines = [nc.vector, nc.scalar, nc.gpsimd]
            n_cast = 0

            with tc.tile_pool(name="wstage", bufs=3) as wstage:
                def load_w(dst, src, kt_count, ncols):
                    nonlocal n_cast
                    for k in range(kt_count):
                        st = wstage.tile([P, HID], f32, tag="wst")
                        nc.sync.dma_start(st[:, :ncols],
                                          src[k * P:(k + 1) * P, :])
                        eng = cast_engines[n_cast % 3]
                        n_cast += 1
                        if eng is nc.scalar:
                            eng.activation(dst[:, k, :], st[:, :ncols], AF.Copy)
                        else:
                            eng.tensor_copy(dst[:, k, :], st[:, :ncols])

                load_w(qkvw, qkvw_d, DT, 3 * D)
                load_w(projw, projw_d, DT, D)
                load_w(fc1w, fc1w_d, DT, HID)
                load_w(fc2w, fc2w_d, HT, D)

            # fold the attention scale into the k columns
            nc.vector.tensor_scalar_mul(qkvw[:, :, D:2 * D],
                                        qkvw[:, :, D:2 * D], 0.125)

            eps_t = singles.tile([P, 1], f32)
            nc.vector.memset(eps_t, EPS)

            qkvb = fc1b = None
            ln1g_r = ln1b_r = ln2g_r = ln2b_r = None
            projb_r = fc2b_r = vb_r = None
            if flags["qkv_b"]:
                qkvb = singles.tile([P, 2 * DT], f32)
                nc.sync.dma_start(
                    qkvb, qkvb_d[:2 * DT * P].rearrange("(t p) -> p t", p=P))
                vb_r = singles.tile([P, D], f32)
                nc.gpsimd.dma_start(vb_r, bcast(qkvb_d[2 * D:]))
            if flags["fc1_b"]:
                fc1b = singles.tile([P, HT], f32)
                nc.sync.dma_start(fc1b, fc1b_d.rearrange("(t p) -> p t", p=P))
            for fl, nmd in (("ln1_g", ln1g_d), ("ln1_b", ln1b_d),
                            ("ln2_g", ln2g_d), ("ln2_b", ln2b_d),
                            ("proj_b", projb_d), ("fc2_b", fc2b_d)):
                if flags[fl]:
                    t_ = singles.tile([P, D], f32, name=f"r_{fl}")
                    nc.gpsimd.dma_start(t_, bcast(nmd))
                    if fl == "ln1_g":
                        ln1g_r = t_
                    elif fl == "ln1_b":
                        ln1b_r = t_
                    elif fl == "ln2_g":
                        ln2g_r = t_
                    elif fl == "ln2_b":
                        ln2b_r = t_
                    elif fl == "proj_b":
                        projb_r = t_
                    else:
                        fc2b_r = t_

            # ---------------- activation pools ----------------
            hTpool = ctx.enter_context(tc.tile_pool(name="hTp", bufs=2))
            qkpool = ctx.enter_context(tc.tile_pool(name="qkp", bufs=1))
            vpool = ctx.enter_context(tc.tile_pool(name="vp", bufs=1))
            bigpool = ctx.enter_context(tc.tile_pool(name="bigp", bufs=2))
            spool = ctx.enter_context(tc.tile_pool(name="sp", bufs=1))
            o1pool = ctx.enter_context(tc.tile_pool(name="o1p", bufs=1))
            xspool = ctx.enter_context(tc.tile_pool(name="xsp", bufs=1))
            hpool = ctx.enter_context(tc.tile_pool(name="hp", bufs=2))
            wapool = ctx.enter_context(tc.tile_pool(name="wap", bufs=1))
            fopool = ctx.enter_context(tc.tile_pool(name="fop", bufs=1))
            statpool = ctx.enter_context(tc.tile_pool(name="stat", bufs=2))

            psb = ctx.enter_context(tc.tile_pool(name="psb", bufs=3,
                                                 space="PSUM"))
            pssc = ctx.enter_context(tc.tile_pool(name="pssc", bufs=2,
                                                  space="PSUM"))
            psav = ctx.enter_context(tc.tile_pool(name="psav", bufs=1,
                                                  space="PSUM"))

            def ln_stats(src, mvs, t):
                stats = statpool.tile([P, 3, 6], f32, tag="bn")
                for c in range(3):
                    nc.vector.bn_stats(stats[:, c, :],
                                       src[:, c * 256:(c + 1) * 256])
                nc.vector.bn_aggr(mvs[:, t, :], stats)

            def ln_rstd(mvs):
                rstd = statpool.tile([P, NT], f32, tag="rstd")
                nc.scalar.activation(rstd, mvs[:, :, 1], AF.Sqrt, bias=eps_t)
                nc.vector.reciprocal(rstd, rstd)
                return rstd

            def ln_norm(src, dst, mvs, rstd, t, g_r, b_r):
                nc.vector.tensor_scalar(dst, src, mvs[:, t, 0:1],
                                        rstd[:, t:t + 1],
                                        op0=OP.subtract, op1=OP.mult)
                if g_r is not None:
                    nc.vector.tensor_tensor(dst, dst, g_r, OP.mult)
                if b_r is not None:
                    nc.vector.tensor_tensor(dst, dst, b_r, OP.add)

            for b in [b_ for _ in range(repeat) for b_ in range(BPC)]:
                # ---- LN1: stats pass, one batched rstd, normalize pass
                mvs1 = statpool.tile([P, NT, 2], f32, tag="mv")
                for t in range(NT):
                    rows = P if t < NT - 1 else LAST_ROWS
                    xs = xspool.tile([P, D], f32, tag="xs", name="xa")
                    if rows < P:
                        nc.vector.memset(xs, 0.0)
                    nc.sync.dma_start(xs[:rows, :], x_d[b, ds(t * P, rows), :])
                    ln_stats(xs, mvs1, t)
                rstd1 = ln_rstd(mvs1)
                hT = hTpool.tile([P, DT, NPH], bf16, tag="hT")
                for t in range(NT):
                    rows = P if t < NT - 1 else LAST_ROWS
                    xs = xspool.tile([P, D], f32, tag="xs", name="xb")
                    if rows < P:
                        nc.vector.memset(xs, 0.0)
                    nc.sync.dma_start(xs[:rows, :], x_d[b, ds(t * P, rows), :])
                    h_t = hpool.tile([P, D], bf16, tag="h")
                    ln_norm(xs, h_t, mvs1, rstd1, t, ln1g_r, ln1b_r)
                    for dt in range(DT):
                        nc.sync.dma_start_transpose(
                            hT[:, dt, ts(t, P)], h_t[:, ts(dt, P)])

                # ---- QKV ----
                # q/k: feature-major, 24 head-slots on partitions 0-63
                qk = qkpool.tile([64, 2 * H, NP], bf16, tag="qk")
                for do in range(2 * DT):          # q,k douts
                    is_k = do >= DT
                    for ncn in range(2):
                        w = TCH[ncn]
                        ps = psb.tile([P, 512], f32, tag="ps", name="psq")
                        for dk in range(DT):
                            nc.tensor.matmul(
                                ps[:, :w], lhsT=qkvw[:, dk, ts(do, P)],
                                rhs=hT[:, dk, ds(TCO[ncn], w)],
                                start=(dk == 0), stop=(dk == DT - 1))
                        t2 = (do - DT) if is_k else do
                        for hh in range(2):
                            slot = (H if is_k else 0) + 2 * t2 + hh
                            dst = qk[:, slot, ds(TCO[ncn], w)]
                            src = ps[hh * 64:hh * 64 + 64, :w]
                            eng = nc.vector if hh == 0 else nc.scalar
                            if qkvb is not None:
                                nc.vector.tensor_scalar(
                                    dst, src,
                                    qkvb[hh * 64:hh * 64 + 64, do:do + 1],
                                    1.0, op0=OP.add, op1=OP.mult)
                            elif eng is nc.scalar:
                                eng.activation(dst, src, AF.Copy)
                            else:
                                eng.tensor_copy(dst, src)

                # v: token-major [tok, h, dh]; pad token rows must be 0
                v_sb = vpool.tile([P, NT, H, DH], bf16, tag="v")
                nc.gpsimd.memset(v_sb[64:, NT - 1, :, :], 0.0)
                for t in range(NT):
                    rows = P if t < NT - 1 else LAST_ROWS
                    for ncn in range(2):
                        ps = psb.tile([P, 512], f32, tag="ps", name="psv")
                        ps = ps[:rows, :384]
                        for dk in range(DT):
                            nc.tensor.matmul(
                                ps, lhsT=hT[:, dk, ds(t * P, rows)],
                                rhs=qkvw[:, dk, ds(2 * D + ncn * 384, 384)],
                                start=(dk == 0), stop=(dk == DT - 1))
                        dst = v_sb[:rows, t, ncn * 6:(ncn + 1) * 6, :]
                        if vb_r is not None:
                            nc.vector.tensor_tensor(
                                dst, ps, vb_r[:rows, ds(ncn * 384, 384)],
                                OP.add)
                        elif t % 2 == 0:
                            nc.scalar.activation(dst, ps, AF.Copy)
                        else:
                            nc.vector.tensor_copy(dst, ps)

                # ---- attention, one query tile at a time ----
                o1 = o1pool.tile([P, NT, D], bf16, tag="o1")
                nc.gpsimd.memset(o1[64:, NT - 1, :], 0.0)
                for qc in range(NT):
                    nq = P if qc < NT - 1 else LAST_W
                    E = bigpool.tile([P, NT, H, P], bf16, tag="big", name="E")
                    # rows of the last k-tile that no scores matmul writes:
                    # set E=1 so S=12 (not 0 -> inf -> NaN); v there is 0.
                    nc.gpsimd.memset(E[64:, NT - 1, :, :nq], 1.0)
                    for kt in range(NT):
                        kw = P if kt < NT - 1 else LAST_W
                        for g in range(3):
                            ps_s = pssc.tile([P, 4, P], f32, name="ps_s")
                            for hh in range(4):
                                h = g * 4 + hh
                                nc.tensor.matmul(
                                    ps_s[:kw, hh, :nq],
                                    lhsT=qk[:, H + h, ds(kt * P, kw)],
                                    rhs=qk[:, h, ds(qc * P, nq)],
                                    start=True, stop=True)
                            nc.scalar.activation(
                                E[:kw, kt, g * 4:g * 4 + 4, :nq],
                                ps_s[:kw, :, :nq], AF.Exp)
                    # S = sum_h E  (bf16 add chain, 2x DVE mode)
                    S = spool.tile([P, NT, P], bf16, tag="S")
                    nc.vector.tensor_tensor(
                        S[:, :, :nq], E[:, :, 0, :nq], E[:, :, 1, :nq], OP.add)
                    for h in range(2, H):
                        nc.vector.tensor_tensor(
                            S[:, :, :nq], S[:, :, :nq], E[:, :, h, :nq],
                            OP.add)
                    with nc.allow_low_precision(reason="softmax denom bf16"):
                        nc.vector.reciprocal(S[:, :, :nq], S[:, :, :nq])
                    nc.vector.tensor_tensor(
                        E[:, :, :, :nq], E[:, :, :, :nq],
                        S[:, :, None, :nq].to_broadcast((P, NT, H, nq)),
                        OP.mult)

                    av = psav.tile([P, DT, P], f32, name="av")
                    for hp in range(DT):
                        for h in (2 * hp, 2 * hp + 1):
                            cb = (h % 2) * 64
                            for kt in range(NT):
                                nc.tensor.matmul(
                                    av[cb:cb + 64, hp, :nq],
                                    lhsT=v_sb[:, kt, h, :],
                                    rhs=E[:, kt, h, :nq],
                                    start=(kt == 0), stop=(kt == NT - 1))
                    wa = wapool.tile([P, DT, P], bf16, tag="wa")
                    nc.scalar.activation(wa[:, :, :nq], av[:, :, :nq], AF.Copy)

                    rows = P if qc < NT - 1 else LAST_ROWS
                    xs2 = xspool.tile([P, D], f32, tag="xs", name="xs2")
                    nc.sync.dma_start(xs2[:rows, :], x_d[b, ds(qc * P, rows), :])
                    for ncn in range(2):
                        ps = psb.tile([P, 512], f32, tag="ps", name="psp")
                        ps = ps[:rows, :384]
                        for dk in range(DT):
                            nc.tensor.matmul(
                                ps, lhsT=wa[:, dk, :rows],
                                rhs=projw[:, dk, ts(ncn, 384)],
                                start=(dk == 0), stop=(dk == DT - 1))
                        dst = o1[:rows, qc, ts(ncn, 384)]
                        nc.vector.tensor_tensor(
                            dst, xs2[:rows, ts(ncn, 384)], ps, OP.add)
                        if projb_r is not None:
                            nc.vector.tensor_tensor(
                                dst, dst, projb_r[:rows, ts(ncn, 384)], OP.add)

                # ---- MLP ----
                # LN2: batched stats from resident o1, then normalize
                mvs2 = statpool.tile([P, NT, 2], f32, tag="mv", name="mvs2")
                for t in range(NT):
                    ln_stats(o1[:, t, :], mvs2, t)
                rstd2 = ln_rstd(mvs2)
                h2T = hTpool.tile([P, DT, NPH], bf16, tag="hT", name="h2T")
                for t in range(NT):
                    h2_t = hpool.tile([P, D], bf16, tag="h", name="h2")
                    ln_norm(o1[:, t, :], h2_t, mvs2, rstd2, t,
                            ln2g_r, ln2b_r)
                    for dt in range(DT):
                        nc.sync.dma_start_transpose(
                            h2T[:, dt, ts(t, P)], h2_t[:, ts(dt, P)])

                # fc1 + gelu over token chunks; gh shares the E pool slots
                ghs = []
                for ncn in range(2):
                    w = TCH[ncn]
                    gh = bigpool.tile([P, HT, TCH[1]], bf16, tag="big",
                                      name="gh")
                    ghs.append(gh)
                    for ht in range(HT):
                        ps = psb.tile([P, 512], f32, tag="ps", name="psf")
                        for dk in range(DT):
                            nc.tensor.matmul(
                                ps[:, :w], lhsT=fc1w[:, dk, ts(ht, P)],
                                rhs=h2T[:, dk, ds(TCO[ncn], w)],
                                start=(dk == 0), stop=(dk == DT - 1))
                        if fc1b is not None:
                            nc.scalar.activation(gh[:, ht, :w], ps[:, :w],
                                                 AF.Gelu,
                                                 bias=fc1b[:, ht:ht + 1])
                        else:
                            nc.scalar.activation(gh[:, ht, :w], ps[:, :w],
                                                 AF.Gelu)

                # fc2 per token tile (t 0,1 from chunk 0; t 2,3,4 from chunk 1)
                for t in range(NT):
                    rows = P if t < NT - 1 else LAST_ROWS
                    # which chunk(s) does tile t live in?
                    fo = fopool.tile([P, D], f32, tag="fo")
                    for ncn in range(2):
                        ps = psb.tile([P, 512], f32, tag="ps", name="ps2")
                        ps = ps[:rows, :384]
                        for kt in range(HT):
                            lo = t * P
                            if lo >= TCH[0]:
                                lhsT = ghs[1][:, kt, ds(lo - TCH[0], rows)]
                            else:
                                lhsT = ghs[0][:, kt, ds(lo, rows)]
                            nc.tensor.matmul(
                                ps, lhsT=lhsT,
                                rhs=fc2w[:, kt, ts(ncn, 384)],
                                start=(kt == 0), stop=(kt == HT - 1))
                        dst = fo[:rows, ts(ncn, 384)]
                        nc.vector.tensor_tensor(
                            dst, o1[:rows, t, ts(ncn, 384)], ps, OP.add)
                        if fc2b_r is not None:
                            nc.vector.tensor_tensor(
                                dst, dst, fc2b_r[:rows, ts(ncn, 384)], OP.add)
                    nc.sync.dma_start(out_d[b, ds(t * P, rows), :],
                                      fo[:rows, :])

    nc.compile()
    return nc


def _flags_from(inputs):
    return {
        "qkv_b": bool(np.any(np.asarray(inputs["qkv_b"]) != 0)),
        "fc1_b": bool(np.any(np.asarray(inputs["fc1_b"]) != 0)),
        "proj_b": bool(np.any(np.asarray(inputs["proj_b"]) != 0)),
        "fc2_b": bool(np.any(np.asarray(inputs["fc2_b"]) != 0)),
        "ln1_g": bool(np.any(np.asarray(inputs["ln1_g"]) != 1)),
        "ln1_b": bool(np.any(np.asarray(inputs["ln1_b"]) != 0)),
        "ln2_g": bool(np.any(np.asarray(inputs["ln2_g"]) != 1)),
        "ln2_b": bool(np.any(np.asarray(inputs["ln2_b"]) != 0)),
    }


def build_nc(inputs, repeat=1):
    flags = _flags_from(inputs)
    key = (tuple(sorted(flags.items())), repeat)
    if key not in _BUILT:
        _BUILT[key] = _build(flags, repeat=repeat)
    return _BUILT[key]


def make_in_maps(inputs):
    full = {k: np.ascontiguousarray(np.asarray(v, dtype=np.float32))
            for k, v in inputs.items()}
    x = full.pop("x")
    in_maps = []
    for c in range(NCORES):
        m = dict(full)
        m["x"] = np.ascontiguousarray(x[c * BPC:(c + 1) * BPC])
        in_maps.append(m)
    return in_maps


def kernel(**inputs):
    global LAST_EXEC_NS
    from concourse import bass_utils

    nc = build_nc(inputs)
    in_maps = make_in_maps(inputs)
    t0 = time.time()
    r = bass_utils.run_bass_kernel_spmd(nc, in_maps,
                                        core_ids=list(range(NCORES)))
    LAST_EXEC_NS = r.exec_time_ns if r.exec_time_ns else int(
        (time.time() - t0) * 1e9)
    out = np.concatenate([r.results[c]["out"] for c in range(NCORES)], axis=0)
    return out.astype(np.float32)
